# revision 17
# baseline (speedup 1.0000x reference)
"""MoE (top-2 of 8 experts, D=768, FF=3072) on 8 Trainium2 NeuronCores.

Strategy: expert-parallel. The router (0.05 GFLOP) runs on host; tokens are
dispatched to their top-2 experts on host, each core runs one expert's FFN
over its routed tokens (the 77 GFLOP that matter), and the host applies the
softmax-weighted combine.

Device layout puts tokens on the matmul free axis, so both matmuls contract
naturally over the partition axis with zero on-device transposes:
    HT[f,t] = relu(sum_d W1[d,f] * XT[d,t] + b1[f])   lhsT=W1, rhs=XT
    YT[d,t] =      sum_f W2[f,d] * HT[f,t] + b2[d]    lhsT=W2, rhs=HT
Inputs are fp16 (well-scaled data; PSUM accumulates fp32), epilogues fp32.
Weights and tokens stream in as slices so the first matmul starts ~12us in
(DMA-ring spin-up ~9us + the 0.75MB crit bundle at the ~230GB/s early rate).

Schedule notes (trace-measured):
- The PE stream is the critical path: ~864 matmuls at ~150-154ns each
  (LDWEIGHTS fully overlaps the previous matmul's streaming; fp8 would be
  ~1.4x faster but e4m3 quantization noise is ~2.6%/operand, over the 2e-2
  error budget).
- MM1 of chunk c+1 is emitted before MM2 of chunk c so the PE always has
  resident-weight work while w2's 4.7MB streams in.
- The final MM2 group is split into two column halves so the last bias-add
  + output DMA overlap the second half's matmuls.
- Exec time ends only after a fixed ~7us postamble (every engine zeroes
  its ~50 semaphores); that and the ~9us DMA-ring spin-up are framework
  overheads this kernel cannot shrink.
"""

import numpy as np

import concourse.tile as tile
from concourse import bacc, mybir
from concourse import bass_utils

D_MODEL = 768
N_EXPERTS = 8
TOP_K = 2
D_FF = 3072
P = 128
KO = D_MODEL // P     # 6   contraction chunks for MM1 / output tiles for MM2
FO = D_FF // P        # 24  output tiles for MM1 / contraction chunks for MM2
FO_PER_W1 = 3         # w1 streams in slices of 3 f-tiles (after the first tile).
                      # Measured optimum: 2-tile slices (12 DMAs) starve the PE
                      # behind per-DMA issue+ramp overhead (177us vs 149us).
W_PARTS = 4           # w2 DMA split: 4 slices of 6 f-tiles each
FO_PER_PART = FO // W_PARTS
WARMUP_MMS = 10       # dummy matmuls during the DMA prologue keep HAM at
WARMUP_N = 512        # 2.4GHz; sized to end right as the crit DMA lands.
                      # Starting real matmuls earlier (smaller warmup and/or
                      # per-ko crit slicing) was measured SLOWER: the early
                      # DMA rate is only ~230GB/s while the rings ramp, the
                      # PE outruns delivery, and stalls >1us trip a HAM
                      # re-throttle window costing 3-7us at half clock.

_program_cache: dict[tuple, object] = {}


def _token_chunks(C):
    """Equal-ish chunks (multiples of 4, <=512) covering C tokens.

    Equal chunks keep the PE's w1 consumption rate matched to the DMA
    delivery rate from the first matmul on (a smaller first chunk starts
    earlier but outruns the weight stream, stalls, and can even trip a
    HAM re-throttle window; measured slower)."""
    nchunks = -(-C // 512)
    base = -(-C // nchunks)
    base = -(-base // 4) * 4
    chunks = []
    t = 0
    while t < C:
        n = min(base, C - t)
        chunks.append((t, n))
        t += n
    return chunks


def _build_program(C):
    """Bass program for one expert's FFN over C routed tokens (SPMD x8)."""
    key = C
    if key in _program_cache:
        return _program_cache[key]

    fp16 = mybir.dt.float16
    fp32 = mybir.dt.float32
    nc = bacc.Bacc("TRN2", target_bir_lowering=False, debug=False,
                   enable_asserts=True, num_devices=N_EXPERTS)

    chunks = _token_chunks(C)
    cmax = max(n for _, n in chunks)

    # DRAM inputs, pre-sliced host-side so every DMA is contiguous per row.
    # Chunk-0 tokens and w1's FIRST f-tile ride in ONE tensor ("crit"):
    # one DMA issue for exactly what the first matmul group needs; the
    # remaining w1 tiles stream in behind (delivery 0.63us/f-tile beats
    # the PE's 0.9us/f-group consumption).
    n0 = chunks[0][1]
    crit_d = nc.dram_tensor("crit", [P, KO, n0 + P], fp16,
                            kind="ExternalInput").ap()
    w1r_d = nc.dram_tensor("w1r", [P, KO, (FO_PER_W1 - 1) * P], fp16,
                           kind="ExternalInput").ap()
    xt_d = [None] + [
        nc.dram_tensor(f"xt{ci}", [P, KO, n], fp16, kind="ExternalInput").ap()
        for ci, (_, n) in list(enumerate(chunks))[1:]]
    w1_d = [None] + [
        nc.dram_tensor(f"w1_{s}", [P, KO, FO_PER_W1 * P], fp16,
                       kind="ExternalInput").ap()
        for s in range(1, FO // FO_PER_W1)]
    w2_d = [nc.dram_tensor(f"w2_{s}", [P, FO_PER_PART, D_MODEL], fp16,
                           kind="ExternalInput").ap() for s in range(W_PARTS)]
    b1_d = nc.dram_tensor("b1c", [P, FO], fp32, kind="ExternalInput").ap()
    b2_d = nc.dram_tensor("b2c", [P, KO], fp32, kind="ExternalInput").ap()
    yt_d = nc.dram_tensor("yt", [P, KO, C], fp32, kind="ExternalOutput").ap()

    with tile.TileContext(nc) as tc:
        with (
            tc.tile_pool(name="wpool", bufs=1) as wpool,
            tc.tile_pool(name="hpool", bufs=2) as hpool,
            tc.tile_pool(name="ypool", bufs=2) as ypool,
            tc.tile_pool(name="pspool", bufs=4, space="PSUM") as pspool,
        ):
            crit_sb = wpool.tile([P, KO, n0 + P], fp16)
            w1r_sb = wpool.tile([P, KO, (FO_PER_W1 - 1) * P], fp16)
            xt_sb = [crit_sb[:, :, :n0]] + [
                wpool.tile([P, KO, n], fp16, name=f"xt_sb{ci}")
                for ci, (_, n) in list(enumerate(chunks))[1:]]
            w1_sb = [
                wpool.tile([P, KO, FO_PER_W1 * P], fp16, name=f"w1_sb{s}")
                for s in range(1, FO // FO_PER_W1)]

            def w1_tile(fo):
                """(tile, local f index) holding w1 f-tile `fo`."""
                if fo == 0:
                    return crit_sb[:, :, n0:], 0
                if fo < FO_PER_W1:
                    return w1r_sb, fo - 1
                return w1_sb[fo // FO_PER_W1 - 1], fo % FO_PER_W1
            w2_sb = [wpool.tile([P, FO_PER_PART, D_MODEL], fp16, name=f"w2_sb{s}")
                     for s in range(W_PARTS)]
            b1_sb = wpool.tile([P, FO], fp32)
            b2_sb = wpool.tile([P, KO], fp32)

            # PE warmup: dummy matmuls on a zeroed tile fill the DMA
            # prologue so the HAM clock-gate reaches 2.4GHz before the
            # real matmuls start.
            warm = wpool.tile([P, WARMUP_N], fp16)
            nc.gpsimd.memset(warm[:], 0.0)
            ps_w = pspool.tile([P, WARMUP_N], fp32, name="ps_w", bufs=1)
            for _ in range(WARMUP_MMS):
                nc.tensor.matmul(ps_w[:], lhsT=warm[:, :P], rhs=warm[:],
                                 start=True, stop=True)

            # Every transfer is split in half across the two HWDGE issue
            # queues (Sync and Scalar), same need-order on both, so both
            # DMA rings pull concurrently from the first byte: the early
            # delivery rate roughly doubles and the critical w1 stream
            # still leads the bulk on each queue. (Putting whole tensors
            # on different queues instead lets the scheduler hoist bulk
            # issues ahead of the critical stream - measured 8.5us worse.)
            def dma2(sb, dr):
                mid = sb.shape[1] // 2
                nc.sync.dma_start(sb[:, :mid], dr[:, :mid])
                nc.scalar.dma_start(sb[:, mid:], dr[:, mid:])

            dma2(crit_sb, crit_d)
            dma2(w1r_sb, w1r_d)
            for s in range(1, FO // FO_PER_W1):
                dma2(w1_sb[s - 1], w1_d[s])
                if s == 1:
                    # b1 (12KB) is not needed until the first epilogue;
                    # issuing it here keeps w1r/w1_1's issue slots early
                    nc.sync.dma_start(b1_sb[:], b1_d[:])
            # x1 leads w2: the schedule below runs MM1(c1) right after
            # MM1(c0), before MM2(c0), so the PE is never waiting on
            # w2's 4.7MB while it still has MM1 work.
            if len(chunks) > 1:
                dma2(xt_sb[1], xt_d[1])
            for s in range(W_PARTS):
                dma2(w2_sb[s], w2_d[s])
            for ci in range(2, len(chunks)):
                dma2(xt_sb[ci], xt_d[ci])
            nc.scalar.dma_start(b2_sb[:], b2_d[:])

            hts = {}

            def mm1(ci):
                t0, nt = chunks[ci]
                ht = hpool.tile([P, FO, cmax], fp16, name="ht")
                hts[ci] = ht
                for fo in range(FO):
                    w1t, f = w1_tile(fo)
                    ps = pspool.tile([P, cmax], fp32, name="ps")
                    for ko in range(KO):
                        nc.tensor.matmul(
                            ps[:, :nt],
                            lhsT=w1t[:, ko, f * P:(f + 1) * P],
                            rhs=xt_sb[ci][:, ko, :nt],
                            start=(ko == 0), stop=(ko == KO - 1),
                        )
                    nc.scalar.activation(
                        ht[:, fo, :nt], ps[:, :nt],
                        mybir.ActivationFunctionType.Relu,
                        bias=b1_sb[:, fo:fo + 1],
                    )


            def mm2(ci):
                t0, nt = chunks[ci]
                ht = hts.pop(ci)
                yt = ypool.tile([P, KO, cmax], fp32, name="yt")
                last = ci == len(chunks) - 1
                for ko in range(KO):
                    # The very last group is split in two column halves so
                    # the first half's bias-add + output DMA overlap the
                    # second half's matmuls, shortening the kernel tail.
                    if last and ko == KO - 1 and nt >= 16:
                        h1 = (nt // 2 + 3) // 4 * 4
                        spans = ((0, h1), (h1, nt - h1))
                    else:
                        spans = ((0, nt),)
                    for c0, cn in spans:
                        ps = pspool.tile([P, cmax], fp32, name="ps")
                        for fo in range(FO):
                            s, f = divmod(fo, FO_PER_PART)
                            nc.tensor.matmul(
                                ps[:, :cn],
                                lhsT=w2_sb[s][:, f, ko * P:(ko + 1) * P],
                                rhs=ht[:, fo, c0:c0 + cn],
                                start=(fo == 0), stop=(fo == FO - 1),
                            )
                        # DVE is ~3x faster than ACT for the plain bias-add
                        # drain; the final one is on the critical tail.
                        nc.vector.tensor_scalar_add(
                            yt[:, ko, c0:c0 + cn], ps[:, :cn],
                            b2_sb[:, ko:ko + 1])
                        nc.sync.dma_start(yt_d[:, ko, t0 + c0:t0 + c0 + cn],
                                          yt[:, ko, c0:c0 + cn])

            # One MM1 stays ahead of the MM2 stream: c0, c1, MM2(c0),
            # c2, MM2(c1), ..., MM2(last).
            mm1(0)
            for c in range(len(chunks)):
                if c + 1 < len(chunks):
                    mm1(c + 1)
                mm2(c)

    nc.compile()
    _program_cache[key] = nc
    return nc


def _route(xf, Wr):
    """Host router: top-2 expert ids + softmax weights (matches lax.top_k)."""
    T = xf.shape[0]
    logits = xf @ Wr
    i1 = np.argmax(logits, axis=1)
    l1 = logits[np.arange(T), i1]
    masked = logits.copy()
    masked[np.arange(T), i1] = -np.inf
    i2 = np.argmax(masked, axis=1)
    l2 = logits[np.arange(T), i2]
    e2 = np.exp((l2 - l1).astype(np.float32))
    wt1 = 1.0 / (1.0 + e2)
    wt2 = e2 / (1.0 + e2)
    return i1, i2, wt1, wt2


def _forward(inputs, trace=False, trace_kwargs=None):
    x = np.ascontiguousarray(np.asarray(inputs["x"], dtype=np.float32))
    Wr = np.asarray(inputs["Wr"], dtype=np.float32)
    W1 = np.asarray(inputs["W1"], dtype=np.float32)
    b1 = np.asarray(inputs["b1"], dtype=np.float32)
    W2 = np.asarray(inputs["W2"], dtype=np.float32)
    b2 = np.asarray(inputs["b2"], dtype=np.float32)

    B, S, D = x.shape
    T = B * S
    xf = x.reshape(T, D)

    i1, i2, wt1, wt2 = _route(xf, Wr)
    idx = [np.nonzero((i1 == e) | (i2 == e))[0] for e in range(N_EXPERTS)]
    gw = [np.where(i1[ix] == e, wt1[ix], wt2[ix]).astype(np.float32)
          for e, ix in enumerate(idx)]

    C = max(-(-max(len(ix) for ix in idx) // 4) * 4, 4)
    nc = _build_program(C)
    chunks = _token_chunks(C)

    in_maps = []
    for e in range(N_EXPERTS):
        ix = idx[e]
        xe = np.zeros((C, D), dtype=np.float16)
        xe[:len(ix)] = xf[ix]
        # XT[d,t] -> [p, ko, t] with d = ko*P + p
        xt = np.ascontiguousarray(xe.T.reshape(KO, P, C).transpose(1, 0, 2))
        w1 = np.ascontiguousarray(
            W1[e].astype(np.float16).reshape(KO, P, D_FF).transpose(1, 0, 2))
        w2 = np.ascontiguousarray(
            W2[e].astype(np.float16).reshape(FO, P, D_MODEL).transpose(1, 0, 2))
        m = {"b1c": np.ascontiguousarray(b1[e].reshape(FO, P).T),
             "b2c": np.ascontiguousarray(b2[e].reshape(KO, P).T)}
        n0 = chunks[0][1]
        m["crit"] = np.ascontiguousarray(
            np.concatenate([xt[:, :, :n0], w1[:, :, :P]], axis=2))
        m["w1r"] = np.ascontiguousarray(w1[:, :, P:FO_PER_W1 * P])
        for ci, (t0, n) in list(enumerate(chunks))[1:]:
            m[f"xt{ci}"] = np.ascontiguousarray(xt[:, :, t0:t0 + n])
        for s in range(1, FO // FO_PER_W1):
            f0 = s * FO_PER_W1 * P
            m[f"w1_{s}"] = np.ascontiguousarray(w1[:, :, f0:f0 + FO_PER_W1 * P])
        for s in range(W_PARTS):
            m[f"w2_{s}"] = np.ascontiguousarray(
                w2[:, s * FO_PER_PART:(s + 1) * FO_PER_PART, :])
        in_maps.append(m)

    try:
        res = bass_utils.run_bass_kernel_spmd(
            nc, in_maps, core_ids=list(range(N_EXPERTS)), trace=trace,
            **(trace_kwargs or {}),
        )
    except Exception:
        # transient device errors (NRT_EXEC_UNIT_UNRECOVERABLE) have been
        # observed once under rapid successive loads; one retry clears them
        res = bass_utils.run_bass_kernel_spmd(
            nc, in_maps, core_ids=list(range(N_EXPERTS)), trace=trace,
            **(trace_kwargs or {}),
        )

    out = np.zeros((T, D), dtype=np.float32)
    for e in range(N_EXPERTS):
        ix = idx[e]
        if len(ix) == 0:
            continue
        # yt [p, ko, t] -> Y [t, d]
        yt = res.results[e]["yt"]
        ye = yt.transpose(2, 1, 0).reshape(C, D)[:len(ix)]
        out[ix] += gw[e][:, None] * ye
    return out.reshape(B, S, D), res


def kernel(**inputs) -> np.ndarray:
    out, _ = _forward(inputs)
    return out



# revision 18
# speedup vs baseline: 1.0998x; 1.0998x over previous
"""MoE (top-2 of 8 experts, D=768, FF=3072) on 8 Trainium2 NeuronCores.

Strategy: expert-parallel. The router (0.05 GFLOP) runs on host; tokens are
dispatched to their top-2 experts on host, each core runs one expert's FFN
over its routed tokens (the 77 GFLOP that matter), and the host applies the
softmax-weighted combine.

Device layout puts tokens on the matmul free axis, so both matmuls contract
naturally over the partition axis with zero on-device transposes:
    HT[f,t] = relu(sum_d W1[d,f] * XT[d,t] + b1[f])   lhsT=W1, rhs=XT
    YT[d,t] =      sum_f W2[f,d] * HT[f,t] + b2[d]    lhsT=W2, rhs=HT
Inputs are fp16 (well-scaled data; PSUM accumulates fp32), epilogues fp32.
Weights and tokens stream in as slices so the first matmul starts ~12us in
(DMA-ring spin-up ~9us + the 0.75MB crit bundle at the ~230GB/s early rate).

Schedule notes (trace-measured):
- The PE stream is the critical path: ~864 matmuls at ~150-154ns each
  (LDWEIGHTS fully overlaps the previous matmul's streaming; fp8 would be
  ~1.4x faster but e4m3 quantization noise is ~2.6%/operand, over the 2e-2
  error budget).
- MM1 of chunk c+1 is emitted before MM2 of chunk c so the PE always has
  resident-weight work while w2's 4.7MB streams in.
- The final MM2 group is split into two column halves so the last bias-add
  + output DMA overlap the second half's matmuls.
- Exec time ends only after a fixed ~7us postamble (every engine zeroes
  its ~50 semaphores); that and the ~9us DMA-ring spin-up are framework
  overheads this kernel cannot shrink.
"""

import numpy as np

import concourse.tile as tile
from concourse import bacc, mybir
from concourse import bass_utils

D_MODEL = 768
N_EXPERTS = 8
TOP_K = 2
D_FF = 3072
P = 128
KO = D_MODEL // P     # 6   contraction chunks for MM1 / output tiles for MM2
FO = D_FF // P        # 24  output tiles for MM1 / contraction chunks for MM2
FO_PER_W1 = 3         # w1 streams in slices of 3 f-tiles (after the first tile).
                      # Measured optimum: 2-tile slices (12 DMAs) starve the PE
                      # behind per-DMA issue+ramp overhead (177us vs 149us).
W_PARTS = 4           # w2 DMA split: 4 slices of 6 f-tiles each
FO_PER_PART = FO // W_PARTS
WARMUP_MMS = 10       # dummy matmuls during the DMA prologue keep HAM at
WARMUP_N = 512        # 2.4GHz; sized to end right as the crit DMA lands.
                      # Starting real matmuls earlier (smaller warmup and/or
                      # per-ko crit slicing) was measured SLOWER: the early
                      # DMA rate is only ~230GB/s while the rings ramp, the
                      # PE outruns delivery, and stalls >1us trip a HAM
                      # re-throttle window costing 3-7us at half clock.

_program_cache: dict[tuple, object] = {}


def _token_chunks(C):
    """Equal-ish chunks (multiples of 4, <=512) covering C tokens.

    Equal chunks keep the PE's w1 consumption rate matched to the DMA
    delivery rate from the first matmul on (a smaller first chunk starts
    earlier but outruns the weight stream, stalls, and can even trip a
    HAM re-throttle window; measured slower)."""
    nchunks = -(-C // 512)
    base = -(-C // nchunks)
    base = -(-base // 4) * 4
    chunks = []
    t = 0
    while t < C:
        n = min(base, C - t)
        chunks.append((t, n))
        t += n
    return chunks


def _build_program(C):
    """Bass program for one expert's FFN over C routed tokens (SPMD x8)."""
    key = C
    if key in _program_cache:
        return _program_cache[key]

    fp16 = mybir.dt.float16
    fp32 = mybir.dt.float32
    nc = bacc.Bacc("TRN2", target_bir_lowering=False, debug=False,
                   enable_asserts=True, num_devices=N_EXPERTS)

    chunks = _token_chunks(C)
    cmax = max(n for _, n in chunks)

    # DRAM inputs, pre-sliced host-side so every DMA is contiguous per row.
    # Chunk-0 tokens and w1's FIRST f-tile ride in ONE tensor ("crit"):
    # one DMA issue for exactly what the first matmul group needs; the
    # remaining w1 tiles stream in behind (delivery 0.63us/f-tile beats
    # the PE's 0.9us/f-group consumption).
    n0 = chunks[0][1]
    crit_d = nc.dram_tensor("crit", [P, KO, n0 + P], fp16,
                            kind="ExternalInput").ap()
    w1r_d = nc.dram_tensor("w1r", [P, KO, (FO_PER_W1 - 1) * P], fp16,
                           kind="ExternalInput").ap()
    xt_d = [None] + [
        nc.dram_tensor(f"xt{ci}", [P, KO, n], fp16, kind="ExternalInput").ap()
        for ci, (_, n) in list(enumerate(chunks))[1:]]
    w1_d = [None] + [
        nc.dram_tensor(f"w1_{s}", [P, KO, FO_PER_W1 * P], fp16,
                       kind="ExternalInput").ap()
        for s in range(1, FO // FO_PER_W1)]
    w2_d = [nc.dram_tensor(f"w2_{s}", [P, FO_PER_PART, D_MODEL], fp16,
                           kind="ExternalInput").ap() for s in range(W_PARTS)]
    b1_d = nc.dram_tensor("b1c", [P, FO], fp32, kind="ExternalInput").ap()
    b2_d = nc.dram_tensor("b2c", [P, KO], fp32, kind="ExternalInput").ap()
    yt_d = nc.dram_tensor("yt", [P, KO, C], fp32, kind="ExternalOutput").ap()

    with tile.TileContext(nc) as tc:
        with (
            tc.tile_pool(name="wpool", bufs=1) as wpool,
            tc.tile_pool(name="hpool", bufs=2) as hpool,
            tc.tile_pool(name="ypool", bufs=2) as ypool,
            tc.tile_pool(name="pspool", bufs=4, space="PSUM") as pspool,
        ):
            crit_sb = wpool.tile([P, KO, n0 + P], fp16)
            w1r_sb = wpool.tile([P, KO, (FO_PER_W1 - 1) * P], fp16)
            xt_sb = [crit_sb[:, :, :n0]] + [
                wpool.tile([P, KO, n], fp16, name=f"xt_sb{ci}")
                for ci, (_, n) in list(enumerate(chunks))[1:]]
            w1_sb = [
                wpool.tile([P, KO, FO_PER_W1 * P], fp16, name=f"w1_sb{s}")
                for s in range(1, FO // FO_PER_W1)]

            def w1_tile(fo):
                """(tile, local f index) holding w1 f-tile `fo`."""
                if fo == 0:
                    return crit_sb[:, :, n0:], 0
                if fo < FO_PER_W1:
                    return w1r_sb, fo - 1
                return w1_sb[fo // FO_PER_W1 - 1], fo % FO_PER_W1
            w2_sb = [wpool.tile([P, FO_PER_PART, D_MODEL], fp16, name=f"w2_sb{s}")
                     for s in range(W_PARTS)]
            b1_sb = wpool.tile([P, FO], fp32)
            b2_sb = wpool.tile([P, KO], fp32)

            # PE warmup: dummy matmuls on a zeroed tile fill the DMA
            # prologue so the HAM clock-gate reaches 2.4GHz before the
            # real matmuls start.
            warm = wpool.tile([P, WARMUP_N], fp16)
            nc.gpsimd.memset(warm[:], 0.0)
            ps_w = pspool.tile([P, WARMUP_N], fp32, name="ps_w", bufs=1)
            for _ in range(WARMUP_MMS):
                nc.tensor.matmul(ps_w[:], lhsT=warm[:, :P], rhs=warm[:],
                                 start=True, stop=True)

            # Every transfer is split in half across the two HWDGE issue
            # queues (Sync and GpSimd), same need-order on both, so both
            # DMA rings pull concurrently from the first byte: the early
            # delivery rate roughly doubles and the critical w1 stream
            # still leads the bulk on each queue. (Putting whole tensors
            # on different queues instead lets the scheduler hoist bulk
            # issues ahead of the critical stream - measured 8.5us worse.
            # The Scalar queue is unusable: its DMA issues serialize with
            # the latency-critical ACT epilogues - measured 16us worse.)
            def dma2(sb, dr):
                mid = sb.shape[1] // 2
                nc.sync.dma_start(sb[:, :mid], dr[:, :mid])
                nc.gpsimd.dma_start(sb[:, mid:], dr[:, mid:])

            dma2(crit_sb, crit_d)
            dma2(w1r_sb, w1r_d)
            for s in range(1, FO // FO_PER_W1):
                dma2(w1_sb[s - 1], w1_d[s])
                if s == 1:
                    # b1 (12KB) is not needed until the first epilogue;
                    # issuing it here keeps w1r/w1_1's issue slots early
                    nc.sync.dma_start(b1_sb[:], b1_d[:])
            # x1 leads w2: the schedule below runs MM1(c1) right after
            # MM1(c0), before MM2(c0), so the PE is never waiting on
            # w2's 4.7MB while it still has MM1 work.
            if len(chunks) > 1:
                dma2(xt_sb[1], xt_d[1])
            for s in range(W_PARTS):
                dma2(w2_sb[s], w2_d[s])
            for ci in range(2, len(chunks)):
                dma2(xt_sb[ci], xt_d[ci])
            nc.gpsimd.dma_start(b2_sb[:], b2_d[:])

            hts = {}

            def mm1(ci):
                t0, nt = chunks[ci]
                ht = hpool.tile([P, FO, cmax], fp16, name="ht")
                hts[ci] = ht
                for fo in range(FO):
                    w1t, f = w1_tile(fo)
                    ps = pspool.tile([P, cmax], fp32, name="ps")
                    for ko in range(KO):
                        nc.tensor.matmul(
                            ps[:, :nt],
                            lhsT=w1t[:, ko, f * P:(f + 1) * P],
                            rhs=xt_sb[ci][:, ko, :nt],
                            start=(ko == 0), stop=(ko == KO - 1),
                        )
                    nc.scalar.activation(
                        ht[:, fo, :nt], ps[:, :nt],
                        mybir.ActivationFunctionType.Relu,
                        bias=b1_sb[:, fo:fo + 1],
                    )


            def mm2(ci):
                t0, nt = chunks[ci]
                ht = hts.pop(ci)
                yt = ypool.tile([P, KO, cmax], fp32, name="yt")
                last = ci == len(chunks) - 1
                for ko in range(KO):
                    # The very last group is split in two column halves so
                    # the first half's bias-add + output DMA overlap the
                    # second half's matmuls, shortening the kernel tail.
                    if last and ko == KO - 1 and nt >= 16:
                        h1 = (nt // 2 + 3) // 4 * 4
                        spans = ((0, h1), (h1, nt - h1))
                    else:
                        spans = ((0, nt),)
                    for c0, cn in spans:
                        ps = pspool.tile([P, cmax], fp32, name="ps")
                        for fo in range(FO):
                            s, f = divmod(fo, FO_PER_PART)
                            nc.tensor.matmul(
                                ps[:, :cn],
                                lhsT=w2_sb[s][:, f, ko * P:(ko + 1) * P],
                                rhs=ht[:, fo, c0:c0 + cn],
                                start=(fo == 0), stop=(fo == FO - 1),
                            )
                        # DVE is ~3x faster than ACT for the plain bias-add
                        # drain; the final one is on the critical tail.
                        nc.vector.tensor_scalar_add(
                            yt[:, ko, c0:c0 + cn], ps[:, :cn],
                            b2_sb[:, ko:ko + 1])
                        nc.sync.dma_start(yt_d[:, ko, t0 + c0:t0 + c0 + cn],
                                          yt[:, ko, c0:c0 + cn])

            # One MM1 stays ahead of the MM2 stream: c0, c1, MM2(c0),
            # c2, MM2(c1), ..., MM2(last).
            mm1(0)
            for c in range(len(chunks)):
                if c + 1 < len(chunks):
                    mm1(c + 1)
                mm2(c)

    nc.compile()
    _program_cache[key] = nc
    return nc


def _route(xf, Wr):
    """Host router: top-2 expert ids + softmax weights (matches lax.top_k)."""
    T = xf.shape[0]
    logits = xf @ Wr
    i1 = np.argmax(logits, axis=1)
    l1 = logits[np.arange(T), i1]
    masked = logits.copy()
    masked[np.arange(T), i1] = -np.inf
    i2 = np.argmax(masked, axis=1)
    l2 = logits[np.arange(T), i2]
    e2 = np.exp((l2 - l1).astype(np.float32))
    wt1 = 1.0 / (1.0 + e2)
    wt2 = e2 / (1.0 + e2)
    return i1, i2, wt1, wt2


def _forward(inputs, trace=False, trace_kwargs=None):
    x = np.ascontiguousarray(np.asarray(inputs["x"], dtype=np.float32))
    Wr = np.asarray(inputs["Wr"], dtype=np.float32)
    W1 = np.asarray(inputs["W1"], dtype=np.float32)
    b1 = np.asarray(inputs["b1"], dtype=np.float32)
    W2 = np.asarray(inputs["W2"], dtype=np.float32)
    b2 = np.asarray(inputs["b2"], dtype=np.float32)

    B, S, D = x.shape
    T = B * S
    xf = x.reshape(T, D)

    i1, i2, wt1, wt2 = _route(xf, Wr)
    idx = [np.nonzero((i1 == e) | (i2 == e))[0] for e in range(N_EXPERTS)]
    gw = [np.where(i1[ix] == e, wt1[ix], wt2[ix]).astype(np.float32)
          for e, ix in enumerate(idx)]

    C = max(-(-max(len(ix) for ix in idx) // 4) * 4, 4)
    nc = _build_program(C)
    chunks = _token_chunks(C)

    in_maps = []
    for e in range(N_EXPERTS):
        ix = idx[e]
        xe = np.zeros((C, D), dtype=np.float16)
        xe[:len(ix)] = xf[ix]
        # XT[d,t] -> [p, ko, t] with d = ko*P + p
        xt = np.ascontiguousarray(xe.T.reshape(KO, P, C).transpose(1, 0, 2))
        w1 = np.ascontiguousarray(
            W1[e].astype(np.float16).reshape(KO, P, D_FF).transpose(1, 0, 2))
        w2 = np.ascontiguousarray(
            W2[e].astype(np.float16).reshape(FO, P, D_MODEL).transpose(1, 0, 2))
        m = {"b1c": np.ascontiguousarray(b1[e].reshape(FO, P).T),
             "b2c": np.ascontiguousarray(b2[e].reshape(KO, P).T)}
        n0 = chunks[0][1]
        m["crit"] = np.ascontiguousarray(
            np.concatenate([xt[:, :, :n0], w1[:, :, :P]], axis=2))
        m["w1r"] = np.ascontiguousarray(w1[:, :, P:FO_PER_W1 * P])
        for ci, (t0, n) in list(enumerate(chunks))[1:]:
            m[f"xt{ci}"] = np.ascontiguousarray(xt[:, :, t0:t0 + n])
        for s in range(1, FO // FO_PER_W1):
            f0 = s * FO_PER_W1 * P
            m[f"w1_{s}"] = np.ascontiguousarray(w1[:, :, f0:f0 + FO_PER_W1 * P])
        for s in range(W_PARTS):
            m[f"w2_{s}"] = np.ascontiguousarray(
                w2[:, s * FO_PER_PART:(s + 1) * FO_PER_PART, :])
        in_maps.append(m)

    try:
        res = bass_utils.run_bass_kernel_spmd(
            nc, in_maps, core_ids=list(range(N_EXPERTS)), trace=trace,
            **(trace_kwargs or {}),
        )
    except Exception:
        # transient device errors (NRT_EXEC_UNIT_UNRECOVERABLE) have been
        # observed once under rapid successive loads; one retry clears them
        res = bass_utils.run_bass_kernel_spmd(
            nc, in_maps, core_ids=list(range(N_EXPERTS)), trace=trace,
            **(trace_kwargs or {}),
        )

    out = np.zeros((T, D), dtype=np.float32)
    for e in range(N_EXPERTS):
        ix = idx[e]
        if len(ix) == 0:
            continue
        # yt [p, ko, t] -> Y [t, d]
        yt = res.results[e]["yt"]
        ye = yt.transpose(2, 1, 0).reshape(C, D)[:len(ix)]
        out[ix] += gw[e][:, None] * ye
    return out.reshape(B, S, D), res


def kernel(**inputs) -> np.ndarray:
    out, _ = _forward(inputs)
    return out



# revision 19
# speedup vs baseline: 1.1109x; 1.0101x over previous
"""MoE (top-2 of 8 experts, D=768, FF=3072) on 8 Trainium2 NeuronCores.

Strategy: expert-parallel. The router (0.05 GFLOP) runs on host; tokens are
dispatched to their top-2 experts on host, each core runs one expert's FFN
over its routed tokens (the 77 GFLOP that matter), and the host applies the
softmax-weighted combine.

Device layout puts tokens on the matmul free axis, so both matmuls contract
naturally over the partition axis with zero on-device transposes:
    HT[f,t] = relu(sum_d W1[d,f] * XT[d,t] + b1[f])   lhsT=W1, rhs=XT
    YT[d,t] =      sum_f W2[f,d] * HT[f,t] + b2[d]    lhsT=W2, rhs=HT
Inputs are fp16 (well-scaled data; PSUM accumulates fp32), epilogues fp32.
Weights and tokens stream in as slices so the first matmul starts ~12us in
(DMA-ring spin-up ~9us + the 0.75MB crit bundle at the ~230GB/s early rate).

Schedule notes (trace-measured):
- The PE stream is the critical path: ~864 matmuls at ~150-154ns each
  (LDWEIGHTS fully overlaps the previous matmul's streaming; fp8 would be
  ~1.4x faster but e4m3 quantization noise is ~2.6%/operand, over the 2e-2
  error budget).
- MM1 of chunk c+1 is emitted before MM2 of chunk c so the PE always has
  resident-weight work while w2's 4.7MB streams in.
- The final MM2 group is split into two column halves so the last bias-add
  + output DMA overlap the second half's matmuls.
- Exec time ends only after a fixed ~7us postamble (every engine zeroes
  its ~50 semaphores); that and the ~9us DMA-ring spin-up are framework
  overheads this kernel cannot shrink.
"""

import numpy as np

import concourse.tile as tile
from concourse import bacc, mybir
from concourse import bass_utils

D_MODEL = 768
N_EXPERTS = 8
TOP_K = 2
D_FF = 3072
P = 128
KO = D_MODEL // P     # 6   contraction chunks for MM1 / output tiles for MM2
FO = D_FF // P        # 24  output tiles for MM1 / contraction chunks for MM2
FO_PER_W1 = 3         # w1 streams in slices of 3 f-tiles (after the first tile).
                      # Measured optimum: 2-tile slices (12 DMAs) starve the PE
                      # behind per-DMA issue+ramp overhead (177us vs 149us).
W_PARTS = 4           # w2 DMA split: 4 slices of 6 f-tiles each
FO_PER_PART = FO // W_PARTS
WARMUP_MMS = 10       # dummy matmuls during the DMA prologue keep HAM at
WARMUP_N = 512        # 2.4GHz; sized to end right as the crit DMA lands.
                      # Starting real matmuls earlier (smaller warmup and/or
                      # per-ko crit slicing) was measured SLOWER: the early
                      # DMA rate is only ~230GB/s while the rings ramp, the
                      # PE outruns delivery, and stalls >1us trip a HAM
                      # re-throttle window costing 3-7us at half clock.

_program_cache: dict[tuple, object] = {}


def _token_chunks(C):
    """Equal-ish chunks (multiples of 4, <=512) covering C tokens.

    Equal chunks keep the PE's w1 consumption rate matched to the DMA
    delivery rate from the first matmul on (a smaller first chunk starts
    earlier but outruns the weight stream, stalls, and can even trip a
    HAM re-throttle window; measured slower)."""
    nchunks = -(-C // 512)
    base = -(-C // nchunks)
    base = -(-base // 4) * 4
    chunks = []
    t = 0
    while t < C:
        n = min(base, C - t)
        chunks.append((t, n))
        t += n
    return chunks


def _build_program(C):
    """Bass program for one expert's FFN over C routed tokens (SPMD x8)."""
    key = C
    if key in _program_cache:
        return _program_cache[key]

    fp16 = mybir.dt.float16
    fp32 = mybir.dt.float32
    nc = bacc.Bacc("TRN2", target_bir_lowering=False, debug=False,
                   enable_asserts=True, num_devices=N_EXPERTS)

    chunks = _token_chunks(C)
    cmax = max(n for _, n in chunks)

    # DRAM inputs, pre-sliced host-side so every DMA is contiguous per row.
    # Chunk-0 tokens and w1's FIRST f-tile ride in ONE tensor ("crit"):
    # one DMA issue for exactly what the first matmul group needs; the
    # remaining w1 tiles stream in behind (delivery 0.63us/f-tile beats
    # the PE's 0.9us/f-group consumption).
    n0 = chunks[0][1]
    crit_d = nc.dram_tensor("crit", [P, KO, n0 + P], fp16,
                            kind="ExternalInput").ap()
    w1r_d = nc.dram_tensor("w1r", [P, KO, (FO_PER_W1 - 1) * P], fp16,
                           kind="ExternalInput").ap()
    xt_d = [None] + [
        nc.dram_tensor(f"xt{ci}", [P, KO, n], fp16, kind="ExternalInput").ap()
        for ci, (_, n) in list(enumerate(chunks))[1:]]
    w1_d = [None] + [
        nc.dram_tensor(f"w1_{s}", [P, KO, FO_PER_W1 * P], fp16,
                       kind="ExternalInput").ap()
        for s in range(1, FO // FO_PER_W1)]
    w2_d = [nc.dram_tensor(f"w2_{s}", [P, FO_PER_PART, D_MODEL], fp16,
                           kind="ExternalInput").ap() for s in range(W_PARTS)]
    b1_d = nc.dram_tensor("b1c", [P, FO], fp32, kind="ExternalInput").ap()
    b2_d = nc.dram_tensor("b2c", [P, KO], fp32, kind="ExternalInput").ap()
    yt_d = nc.dram_tensor("yt", [P, KO, C], fp32, kind="ExternalOutput").ap()

    with tile.TileContext(nc) as tc:
        with (
            tc.tile_pool(name="wpool", bufs=1) as wpool,
            tc.tile_pool(name="hpool", bufs=2) as hpool,
            tc.tile_pool(name="ypool", bufs=2) as ypool,
            tc.tile_pool(name="pspool", bufs=4, space="PSUM") as pspool,
        ):
            crit_sb = wpool.tile([P, KO, n0 + P], fp16)
            w1r_sb = wpool.tile([P, KO, (FO_PER_W1 - 1) * P], fp16)
            xt_sb = [crit_sb[:, :, :n0]] + [
                wpool.tile([P, KO, n], fp16, name=f"xt_sb{ci}")
                for ci, (_, n) in list(enumerate(chunks))[1:]]
            w1_sb = [
                wpool.tile([P, KO, FO_PER_W1 * P], fp16, name=f"w1_sb{s}")
                for s in range(1, FO // FO_PER_W1)]

            def w1_tile(fo):
                """(tile, local f index) holding w1 f-tile `fo`."""
                if fo == 0:
                    return crit_sb[:, :, n0:], 0
                if fo < FO_PER_W1:
                    return w1r_sb, fo - 1
                return w1_sb[fo // FO_PER_W1 - 1], fo % FO_PER_W1
            w2_sb = [wpool.tile([P, FO_PER_PART, D_MODEL], fp16, name=f"w2_sb{s}")
                     for s in range(W_PARTS)]
            b1_sb = wpool.tile([P, FO], fp32)
            b2_sb = wpool.tile([P, KO], fp32)

            # PE warmup: dummy matmuls on a zeroed tile fill the DMA
            # prologue so the HAM clock-gate reaches 2.4GHz before the
            # real matmuls start.
            warm = wpool.tile([P, WARMUP_N], fp16)
            nc.gpsimd.memset(warm[:], 0.0)
            ps_w = pspool.tile([P, WARMUP_N], fp32, name="ps_w", bufs=1)
            for _ in range(WARMUP_MMS):
                nc.tensor.matmul(ps_w[:], lhsT=warm[:, :P], rhs=warm[:],
                                 start=True, stop=True)

            # DMA order = need order, ALL on the Sync HWDGE queue: the
            # crit bundle (chunk-0 tokens + first w1 f-tile) unblocks the
            # first matmuls; the rest of w1, then x1, w2, x2 stream in
            # behind. Multi-queue variants all measured worse: the Scalar
            # queue's issues serialize ahead of the latency-critical ACT
            # epilogues (+16us), and the GpSimd queue issues descriptors
            # too slowly (~0.7us each, after its preamble) so a mirrored
            # half-split delays the crit arrival (+2us).
            nc.sync.dma_start(crit_sb[:], crit_d[:])
            nc.sync.dma_start(w1r_sb[:], w1r_d[:])
            for s in range(1, FO // FO_PER_W1):
                nc.sync.dma_start(w1_sb[s - 1][:], w1_d[s][:])
                if s == 1:
                    # b1 (12KB) is not needed until the first epilogue;
                    # issuing it here keeps w1r/w1_1's issue slots early
                    nc.sync.dma_start(b1_sb[:], b1_d[:])
            # x1 leads w2: the schedule below runs MM1(c1) right after
            # MM1(c0), before MM2(c0), so the PE is never waiting on
            # w2's 4.7MB while it still has MM1 work.
            if len(chunks) > 1:
                nc.sync.dma_start(xt_sb[1][:], xt_d[1][:])
            for s in range(W_PARTS):
                nc.sync.dma_start(w2_sb[s][:], w2_d[s][:])
            for ci in range(2, len(chunks)):
                nc.sync.dma_start(xt_sb[ci][:], xt_d[ci][:])
            nc.sync.dma_start(b2_sb[:], b2_d[:])

            hts = {}

            def mm1(ci):
                t0, nt = chunks[ci]
                ht = hpool.tile([P, FO, cmax], fp16, name="ht")
                hts[ci] = ht
                for fo in range(FO):
                    w1t, f = w1_tile(fo)
                    ps = pspool.tile([P, cmax], fp32, name="ps")
                    for ko in range(KO):
                        nc.tensor.matmul(
                            ps[:, :nt],
                            lhsT=w1t[:, ko, f * P:(f + 1) * P],
                            rhs=xt_sb[ci][:, ko, :nt],
                            start=(ko == 0), stop=(ko == KO - 1),
                        )
                    nc.scalar.activation(
                        ht[:, fo, :nt], ps[:, :nt],
                        mybir.ActivationFunctionType.Relu,
                        bias=b1_sb[:, fo:fo + 1],
                    )


            def mm2(ci):
                t0, nt = chunks[ci]
                ht = hts.pop(ci)
                yt = ypool.tile([P, KO, cmax], fp32, name="yt")
                last = ci == len(chunks) - 1
                for ko in range(KO):
                    # The very last group is split in two column halves so
                    # the first half's bias-add + output DMA overlap the
                    # second half's matmuls, shortening the kernel tail.
                    if last and ko == KO - 1 and nt >= 16:
                        h1 = (nt // 2 + 3) // 4 * 4
                        spans = ((0, h1), (h1, nt - h1))
                    else:
                        spans = ((0, nt),)
                    for c0, cn in spans:
                        ps = pspool.tile([P, cmax], fp32, name="ps")
                        for fo in range(FO):
                            s, f = divmod(fo, FO_PER_PART)
                            nc.tensor.matmul(
                                ps[:, :cn],
                                lhsT=w2_sb[s][:, f, ko * P:(ko + 1) * P],
                                rhs=ht[:, fo, c0:c0 + cn],
                                start=(fo == 0), stop=(fo == FO - 1),
                            )
                        # DVE is ~3x faster than ACT for the plain bias-add
                        # drain; the final one is on the critical tail.
                        nc.vector.tensor_scalar_add(
                            yt[:, ko, c0:c0 + cn], ps[:, :cn],
                            b2_sb[:, ko:ko + 1])
                        nc.sync.dma_start(yt_d[:, ko, t0 + c0:t0 + c0 + cn],
                                          yt[:, ko, c0:c0 + cn])

            # One MM1 stays ahead of the MM2 stream: c0, c1, MM2(c0),
            # c2, MM2(c1), ..., MM2(last).
            mm1(0)
            for c in range(len(chunks)):
                if c + 1 < len(chunks):
                    mm1(c + 1)
                mm2(c)

    nc.compile()
    _program_cache[key] = nc
    return nc


def _route(xf, Wr):
    """Host router: top-2 expert ids + softmax weights (matches lax.top_k)."""
    T = xf.shape[0]
    logits = xf @ Wr
    i1 = np.argmax(logits, axis=1)
    l1 = logits[np.arange(T), i1]
    masked = logits.copy()
    masked[np.arange(T), i1] = -np.inf
    i2 = np.argmax(masked, axis=1)
    l2 = logits[np.arange(T), i2]
    e2 = np.exp((l2 - l1).astype(np.float32))
    wt1 = 1.0 / (1.0 + e2)
    wt2 = e2 / (1.0 + e2)
    return i1, i2, wt1, wt2


def _forward(inputs, trace=False, trace_kwargs=None):
    x = np.ascontiguousarray(np.asarray(inputs["x"], dtype=np.float32))
    Wr = np.asarray(inputs["Wr"], dtype=np.float32)
    W1 = np.asarray(inputs["W1"], dtype=np.float32)
    b1 = np.asarray(inputs["b1"], dtype=np.float32)
    W2 = np.asarray(inputs["W2"], dtype=np.float32)
    b2 = np.asarray(inputs["b2"], dtype=np.float32)

    B, S, D = x.shape
    T = B * S
    xf = x.reshape(T, D)

    i1, i2, wt1, wt2 = _route(xf, Wr)
    idx = [np.nonzero((i1 == e) | (i2 == e))[0] for e in range(N_EXPERTS)]
    gw = [np.where(i1[ix] == e, wt1[ix], wt2[ix]).astype(np.float32)
          for e, ix in enumerate(idx)]

    C = max(-(-max(len(ix) for ix in idx) // 4) * 4, 4)
    nc = _build_program(C)
    chunks = _token_chunks(C)

    in_maps = []
    for e in range(N_EXPERTS):
        ix = idx[e]
        xe = np.zeros((C, D), dtype=np.float16)
        xe[:len(ix)] = xf[ix]
        # XT[d,t] -> [p, ko, t] with d = ko*P + p
        xt = np.ascontiguousarray(xe.T.reshape(KO, P, C).transpose(1, 0, 2))
        w1 = np.ascontiguousarray(
            W1[e].astype(np.float16).reshape(KO, P, D_FF).transpose(1, 0, 2))
        w2 = np.ascontiguousarray(
            W2[e].astype(np.float16).reshape(FO, P, D_MODEL).transpose(1, 0, 2))
        m = {"b1c": np.ascontiguousarray(b1[e].reshape(FO, P).T),
             "b2c": np.ascontiguousarray(b2[e].reshape(KO, P).T)}
        n0 = chunks[0][1]
        m["crit"] = np.ascontiguousarray(
            np.concatenate([xt[:, :, :n0], w1[:, :, :P]], axis=2))
        m["w1r"] = np.ascontiguousarray(w1[:, :, P:FO_PER_W1 * P])
        for ci, (t0, n) in list(enumerate(chunks))[1:]:
            m[f"xt{ci}"] = np.ascontiguousarray(xt[:, :, t0:t0 + n])
        for s in range(1, FO // FO_PER_W1):
            f0 = s * FO_PER_W1 * P
            m[f"w1_{s}"] = np.ascontiguousarray(w1[:, :, f0:f0 + FO_PER_W1 * P])
        for s in range(W_PARTS):
            m[f"w2_{s}"] = np.ascontiguousarray(
                w2[:, s * FO_PER_PART:(s + 1) * FO_PER_PART, :])
        in_maps.append(m)

    try:
        res = bass_utils.run_bass_kernel_spmd(
            nc, in_maps, core_ids=list(range(N_EXPERTS)), trace=trace,
            **(trace_kwargs or {}),
        )
    except Exception:
        # transient device errors (NRT_EXEC_UNIT_UNRECOVERABLE) have been
        # observed once under rapid successive loads; one retry clears them
        res = bass_utils.run_bass_kernel_spmd(
            nc, in_maps, core_ids=list(range(N_EXPERTS)), trace=trace,
            **(trace_kwargs or {}),
        )

    out = np.zeros((T, D), dtype=np.float32)
    for e in range(N_EXPERTS):
        ix = idx[e]
        if len(ix) == 0:
            continue
        # yt [p, ko, t] -> Y [t, d]
        yt = res.results[e]["yt"]
        ye = yt.transpose(2, 1, 0).reshape(C, D)[:len(ix)]
        out[ix] += gw[e][:, None] * ye
    return out.reshape(B, S, D), res


def kernel(**inputs) -> np.ndarray:
    out, _ = _forward(inputs)
    return out



# revision 24
# speedup vs baseline: 1.1151x; 1.0038x over previous
"""MoE (top-2 of 8 experts, D=768, FF=3072) on 8 Trainium2 NeuronCores.

Strategy: expert-parallel. The router (0.05 GFLOP) runs on host; tokens are
dispatched to their top-2 experts on host, each core runs one expert's FFN
over its routed tokens (the 77 GFLOP that matter), and the host applies the
softmax-weighted combine.

Device layout puts tokens on the matmul free axis, so both matmuls contract
naturally over the partition axis with zero on-device transposes:
    HT[f,t] = relu(sum_d W1[d,f] * XT[d,t] + b1[f])   lhsT=W1, rhs=XT
    YT[d,t] =      sum_f W2[f,d] * HT[f,t] + b2[d]    lhsT=W2, rhs=HT
Inputs are fp16 (well-scaled data; PSUM accumulates fp32), epilogues fp32.
Weights and tokens stream in as slices so the first matmul starts ~12us in
(DMA-ring spin-up ~9us + the 0.75MB crit bundle at the ~230GB/s early rate).

Schedule notes (trace-measured):
- The PE stream is the critical path: ~864 matmuls at ~150-154ns each
  (LDWEIGHTS fully overlaps the previous matmul's streaming; fp8 would be
  ~1.4x faster but e4m3 quantization noise is ~2.6%/operand, over the 2e-2
  error budget).
- MM1 of chunk c+1 is emitted before MM2 of chunk c so the PE always has
  resident-weight work while w2's 4.7MB streams in.
- The final MM2 group is split into two column halves so the last bias-add
  + output DMA overlap the second half's matmuls.
- Exec time ends only after a fixed ~7us postamble (every engine zeroes
  its ~50 semaphores); that and the ~9us DMA-ring spin-up are framework
  overheads this kernel cannot shrink.
"""

import numpy as np

import concourse.tile as tile
from concourse import bacc, mybir
from concourse import bass_utils

D_MODEL = 768
N_EXPERTS = 8
TOP_K = 2
D_FF = 3072
P = 128
KO = D_MODEL // P     # 6   contraction chunks for MM1 / output tiles for MM2
FO = D_FF // P        # 24  output tiles for MM1 / contraction chunks for MM2
FO_PER_W1 = 3         # f-tiles covered by crit(1) + w1r(2)
# Remaining 21 w1 f-tiles stream in 7 slices, front-loaded finer: the
# early slices are on the PE's critical path while the DMA rings are
# still ramping (~230GB/s), so smaller first quanta land sooner and
# shrink the fo3-8 weight-wait gap. Same 8-issue total as uniform 3s
# (12 uniform 2-tile DMAs measured slower: per-DMA issue+ramp overhead).
W1_SLICES = [2, 2, 3, 3, 3, 4, 4]
W1_OFFS = [FO_PER_W1]
for _n in W1_SLICES:
    W1_OFFS.append(W1_OFFS[-1] + _n)
W_PARTS = 4           # w2 DMA split: 4 slices of 6 f-tiles each
FO_PER_PART = FO // W_PARTS
WARMUP_MMS = 10       # dummy matmuls during the DMA prologue keep HAM at
WARMUP_N = 512        # 2.4GHz; sized to end right as the crit DMA lands.
                      # Starting real matmuls earlier (smaller warmup and/or
                      # per-ko crit slicing) was measured SLOWER: the early
                      # DMA rate is only ~230GB/s while the rings ramp, the
                      # PE outruns delivery, and stalls >1us trip a HAM
                      # re-throttle window costing 3-7us at half clock.

_program_cache: dict[tuple, object] = {}


def _token_chunks(C):
    """Equal-ish chunks (multiples of 4, <=512) covering C tokens.

    Equal chunks keep the PE's w1 consumption rate matched to the DMA
    delivery rate from the first matmul on (a smaller first chunk starts
    earlier but outruns the weight stream, stalls, and can even trip a
    HAM re-throttle window; measured slower)."""
    nchunks = -(-C // 512)
    base = -(-C // nchunks)
    base = -(-base // 4) * 4
    chunks = []
    t = 0
    while t < C:
        n = min(base, C - t)
        chunks.append((t, n))
        t += n
    return chunks


def _build_program(C):
    """Bass program for one expert's FFN over C routed tokens (SPMD x8)."""
    key = C
    if key in _program_cache:
        return _program_cache[key]

    fp16 = mybir.dt.float16
    fp32 = mybir.dt.float32
    nc = bacc.Bacc("TRN2", target_bir_lowering=False, debug=False,
                   enable_asserts=True, num_devices=N_EXPERTS)

    chunks = _token_chunks(C)
    cmax = max(n for _, n in chunks)

    # DRAM inputs, pre-sliced host-side so every DMA is contiguous per row.
    # Chunk-0 tokens and w1's FIRST f-tile ride in ONE tensor ("crit"):
    # one DMA issue for exactly what the first matmul group needs; the
    # remaining w1 tiles stream in behind (delivery 0.63us/f-tile beats
    # the PE's 0.9us/f-group consumption).
    n0 = chunks[0][1]
    crit_d = nc.dram_tensor("crit", [P, KO, n0 + P], fp16,
                            kind="ExternalInput").ap()
    w1r_d = nc.dram_tensor("w1r", [P, KO, (FO_PER_W1 - 1) * P], fp16,
                           kind="ExternalInput").ap()
    xt_d = [None] + [
        nc.dram_tensor(f"xt{ci}", [P, KO, n], fp16, kind="ExternalInput").ap()
        for ci, (_, n) in list(enumerate(chunks))[1:]]
    w1_d = [
        nc.dram_tensor(f"w1_{s}", [P, KO, n * P], fp16,
                       kind="ExternalInput").ap()
        for s, n in enumerate(W1_SLICES)]
    w2_d = [nc.dram_tensor(f"w2_{s}", [P, FO_PER_PART, D_MODEL], fp16,
                           kind="ExternalInput").ap() for s in range(W_PARTS)]
    b1_d = nc.dram_tensor("b1c", [P, FO], fp32, kind="ExternalInput").ap()
    b2_d = nc.dram_tensor("b2c", [P, KO], fp32, kind="ExternalInput").ap()
    yt_d = nc.dram_tensor("yt", [P, KO, C], fp32, kind="ExternalOutput").ap()

    with tile.TileContext(nc) as tc:
        with (
            tc.tile_pool(name="wpool", bufs=1) as wpool,
            tc.tile_pool(name="hpool", bufs=2) as hpool,
            tc.tile_pool(name="ypool", bufs=2) as ypool,
            tc.tile_pool(name="pspool", bufs=4, space="PSUM") as pspool,
        ):
            crit_sb = wpool.tile([P, KO, n0 + P], fp16)
            w1r_sb = wpool.tile([P, KO, (FO_PER_W1 - 1) * P], fp16)
            xt_sb = [crit_sb[:, :, :n0]] + [
                wpool.tile([P, KO, n], fp16, name=f"xt_sb{ci}")
                for ci, (_, n) in list(enumerate(chunks))[1:]]
            w1_sb = [
                wpool.tile([P, KO, n * P], fp16, name=f"w1_sb{s}")
                for s, n in enumerate(W1_SLICES)]

            def w1_tile(fo):
                """(tile, local f index) holding w1 f-tile `fo`."""
                if fo == 0:
                    return crit_sb[:, :, n0:], 0
                if fo < FO_PER_W1:
                    return w1r_sb, fo - 1
                s = 0
                while W1_OFFS[s + 1] <= fo:
                    s += 1
                return w1_sb[s], fo - W1_OFFS[s]
            w2_sb = [wpool.tile([P, FO_PER_PART, D_MODEL], fp16, name=f"w2_sb{s}")
                     for s in range(W_PARTS)]
            b1_sb = wpool.tile([P, FO], fp32)
            b2_sb = wpool.tile([P, KO], fp32)

            # PE warmup: dummy matmuls on a zeroed tile fill the DMA
            # prologue so the HAM clock-gate reaches 2.4GHz before the
            # real matmuls start.
            warm = wpool.tile([P, WARMUP_N], fp16)
            nc.gpsimd.memset(warm[:], 0.0)
            ps_w = pspool.tile([P, WARMUP_N], fp32, name="ps_w", bufs=1)
            for _ in range(WARMUP_MMS):
                nc.tensor.matmul(ps_w[:], lhsT=warm[:, :P], rhs=warm[:],
                                 start=True, stop=True)

            # DMA order = need order, ALL on the Sync HWDGE queue: the
            # crit bundle (chunk-0 tokens + first w1 f-tile) unblocks the
            # first matmuls; the rest of w1, then x1, w2, x2 stream in
            # behind. Multi-queue variants all measured worse: the Scalar
            # queue's issues serialize ahead of the latency-critical ACT
            # epilogues (+16us), and the GpSimd queue issues descriptors
            # too slowly (~0.7us each, after its preamble) so a mirrored
            # half-split delays the crit arrival (+2us).
            nc.sync.dma_start(crit_sb[:], crit_d[:])
            nc.sync.dma_start(w1r_sb[:], w1r_d[:])
            for s in range(len(W1_SLICES)):
                nc.sync.dma_start(w1_sb[s][:], w1_d[s][:])
                if s == 0:
                    # b1 (12KB) is not needed until the first epilogue;
                    # issuing it here keeps w1r/w1_0's issue slots early
                    nc.sync.dma_start(b1_sb[:], b1_d[:])
            # x1 leads w2: the schedule below runs MM1(c1) right after
            # MM1(c0), before MM2(c0), so the PE is never waiting on
            # w2's 4.7MB while it still has MM1 work.
            if len(chunks) > 1:
                nc.sync.dma_start(xt_sb[1][:], xt_d[1][:])
            for s in range(W_PARTS):
                nc.sync.dma_start(w2_sb[s][:], w2_d[s][:])
            for ci in range(2, len(chunks)):
                nc.sync.dma_start(xt_sb[ci][:], xt_d[ci][:])
            nc.sync.dma_start(b2_sb[:], b2_d[:])

            hts = {}

            def mm1(ci):
                t0, nt = chunks[ci]
                ht = hpool.tile([P, FO, cmax], fp16, name="ht")
                hts[ci] = ht
                for fo in range(FO):
                    w1t, f = w1_tile(fo)
                    ps = pspool.tile([P, cmax], fp32, name="ps")
                    for ko in range(KO):
                        nc.tensor.matmul(
                            ps[:, :nt],
                            lhsT=w1t[:, ko, f * P:(f + 1) * P],
                            rhs=xt_sb[ci][:, ko, :nt],
                            start=(ko == 0), stop=(ko == KO - 1),
                        )
                    nc.scalar.activation(
                        ht[:, fo, :nt], ps[:, :nt],
                        mybir.ActivationFunctionType.Relu,
                        bias=b1_sb[:, fo:fo + 1],
                    )


            def mm2(ci):
                t0, nt = chunks[ci]
                ht = hts.pop(ci)
                yt = ypool.tile([P, KO, cmax], fp32, name="yt")
                last = ci == len(chunks) - 1
                for ko in range(KO):
                    # The very last group is split in two column halves so
                    # the first half's bias-add + output DMA overlap the
                    # second half's matmuls, shortening the kernel tail.
                    if last and ko == KO - 1 and nt >= 16:
                        h1 = (nt // 2 + 3) // 4 * 4
                        spans = ((0, h1), (h1, nt - h1))
                    else:
                        spans = ((0, nt),)
                    for c0, cn in spans:
                        ps = pspool.tile([P, cmax], fp32, name="ps")
                        for fo in range(FO):
                            s, f = divmod(fo, FO_PER_PART)
                            nc.tensor.matmul(
                                ps[:, :cn],
                                lhsT=w2_sb[s][:, f, ko * P:(ko + 1) * P],
                                rhs=ht[:, fo, c0:c0 + cn],
                                start=(fo == 0), stop=(fo == FO - 1),
                            )
                        # DVE is ~3x faster than ACT for the plain bias-add
                        # drain; the final one is on the critical tail.
                        nc.vector.tensor_scalar_add(
                            yt[:, ko, c0:c0 + cn], ps[:, :cn],
                            b2_sb[:, ko:ko + 1])
                        nc.sync.dma_start(yt_d[:, ko, t0 + c0:t0 + c0 + cn],
                                          yt[:, ko, c0:c0 + cn])

            # One MM1 stays ahead of the MM2 stream: c0, c1, MM2(c0),
            # c2, MM2(c1), ..., MM2(last).
            mm1(0)
            for c in range(len(chunks)):
                if c + 1 < len(chunks):
                    mm1(c + 1)
                mm2(c)

    nc.compile()
    _program_cache[key] = nc
    return nc


def _route(xf, Wr):
    """Host router: top-2 expert ids + softmax weights (matches lax.top_k)."""
    T = xf.shape[0]
    logits = xf @ Wr
    i1 = np.argmax(logits, axis=1)
    l1 = logits[np.arange(T), i1]
    masked = logits.copy()
    masked[np.arange(T), i1] = -np.inf
    i2 = np.argmax(masked, axis=1)
    l2 = logits[np.arange(T), i2]
    e2 = np.exp((l2 - l1).astype(np.float32))
    wt1 = 1.0 / (1.0 + e2)
    wt2 = e2 / (1.0 + e2)
    return i1, i2, wt1, wt2


def _forward(inputs, trace=False, trace_kwargs=None):
    x = np.ascontiguousarray(np.asarray(inputs["x"], dtype=np.float32))
    Wr = np.asarray(inputs["Wr"], dtype=np.float32)
    W1 = np.asarray(inputs["W1"], dtype=np.float32)
    b1 = np.asarray(inputs["b1"], dtype=np.float32)
    W2 = np.asarray(inputs["W2"], dtype=np.float32)
    b2 = np.asarray(inputs["b2"], dtype=np.float32)

    B, S, D = x.shape
    T = B * S
    xf = x.reshape(T, D)

    i1, i2, wt1, wt2 = _route(xf, Wr)
    idx = [np.nonzero((i1 == e) | (i2 == e))[0] for e in range(N_EXPERTS)]
    gw = [np.where(i1[ix] == e, wt1[ix], wt2[ix]).astype(np.float32)
          for e, ix in enumerate(idx)]

    C = max(-(-max(len(ix) for ix in idx) // 4) * 4, 4)
    nc = _build_program(C)
    chunks = _token_chunks(C)

    in_maps = []
    for e in range(N_EXPERTS):
        ix = idx[e]
        xe = np.zeros((C, D), dtype=np.float16)
        xe[:len(ix)] = xf[ix]
        # XT[d,t] -> [p, ko, t] with d = ko*P + p
        xt = np.ascontiguousarray(xe.T.reshape(KO, P, C).transpose(1, 0, 2))
        w1 = np.ascontiguousarray(
            W1[e].astype(np.float16).reshape(KO, P, D_FF).transpose(1, 0, 2))
        w2 = np.ascontiguousarray(
            W2[e].astype(np.float16).reshape(FO, P, D_MODEL).transpose(1, 0, 2))
        m = {"b1c": np.ascontiguousarray(b1[e].reshape(FO, P).T),
             "b2c": np.ascontiguousarray(b2[e].reshape(KO, P).T)}
        n0 = chunks[0][1]
        m["crit"] = np.ascontiguousarray(
            np.concatenate([xt[:, :, :n0], w1[:, :, :P]], axis=2))
        m["w1r"] = np.ascontiguousarray(w1[:, :, P:FO_PER_W1 * P])
        for ci, (t0, n) in list(enumerate(chunks))[1:]:
            m[f"xt{ci}"] = np.ascontiguousarray(xt[:, :, t0:t0 + n])
        for s, ntile in enumerate(W1_SLICES):
            f0 = W1_OFFS[s] * P
            m[f"w1_{s}"] = np.ascontiguousarray(w1[:, :, f0:f0 + ntile * P])
        for s in range(W_PARTS):
            m[f"w2_{s}"] = np.ascontiguousarray(
                w2[:, s * FO_PER_PART:(s + 1) * FO_PER_PART, :])
        in_maps.append(m)

    try:
        res = bass_utils.run_bass_kernel_spmd(
            nc, in_maps, core_ids=list(range(N_EXPERTS)), trace=trace,
            **(trace_kwargs or {}),
        )
    except Exception:
        # transient device errors (NRT_EXEC_UNIT_UNRECOVERABLE) have been
        # observed once under rapid successive loads; one retry clears them
        res = bass_utils.run_bass_kernel_spmd(
            nc, in_maps, core_ids=list(range(N_EXPERTS)), trace=trace,
            **(trace_kwargs or {}),
        )

    out = np.zeros((T, D), dtype=np.float32)
    for e in range(N_EXPERTS):
        ix = idx[e]
        if len(ix) == 0:
            continue
        # yt [p, ko, t] -> Y [t, d]
        yt = res.results[e]["yt"]
        ye = yt.transpose(2, 1, 0).reshape(C, D)[:len(ix)]
        out[ix] += gw[e][:, None] * ye
    return out.reshape(B, S, D), res


def kernel(**inputs) -> np.ndarray:
    out, _ = _forward(inputs)
    return out

